# revision 2
# baseline (speedup 1.0000x reference)
"""AiLUT kernel for Trainium2 (8 NeuronCores, data-parallel).

Host side computes the tiny backbone (resize->convs->IN->heads) exactly as the
reference does (150 MFLOP on 2x3x256x256 -- negligible), producing per-batch
LUTs + vertices, then builds an expanded per-cell corner table E:
  E[cell][c*8 + db*4 + dg*2 + dr] = lut[c, ib+db, ig+dg, ir+dr]
  E[cell][24..29] = (vr0, rr, vg0, rg, vb0, rb)   (for exact frac computation)
for cell = (ib*32 + ig)*32 + ir, 32 f32 per row (128 B).

The device (Bass, SPMD on 8 cores; each core owns one (batch, row-block)
quarter = 1,048,576 pixels) does the memory-bound transform:
  - searchsorted via 32 fused is_ge+add ops per channel (vertex thresholds
    come in via a small per-core tensor, so one program serves all cores)
  - cell index build + f32->i32 convert
  - per-pixel-column [128,1] indirect-DMA row gather from E
  - exact frac + trilinear reduce on DVE
"""

import numpy as np

V = 33
EPS = 1e-5

# ----------------------------------------------------------------- host math


def _resize_bilinear_np(x):
    # 2048 -> 256, align_corners=False: src = (i+0.5)*8-0.5 -> i0 = 8i+3, f=0.5
    b, c, H, W = x.shape
    y0 = np.arange(256) * 8 + 3
    rows = x[:, :, y0, :] * 0.5 + x[:, :, y0 + 1, :] * 0.5
    return rows[:, :, :, y0] * 0.5 + rows[:, :, :, y0 + 1] * 0.5


def _conv_s2_np(x, w, bias):
    # x [b,ci,H,W] -> pad 1, stride 2, 3x3
    b, ci, H, W = x.shape
    co = w.shape[0]
    oh, ow = H // 2, W // 2
    xp = np.zeros((b, ci, H + 2, W + 2), np.float32)
    xp[:, :, 1 : H + 1, 1 : W + 1] = x
    y = np.zeros((b, co, oh, ow), np.float32)
    for ky in range(3):
        for kx in range(3):
            xs = xp[:, :, ky : ky + 2 * oh : 2, kx : kx + 2 * ow : 2]
            y += np.einsum("oi,biyx->boyx", w[:, :, ky, kx], xs,
                           dtype=np.float32, casting="same_kind")
    return y + bias[None, :, None, None]


def _lrelu_np(x):
    return np.where(x >= 0, x, np.float32(0.2) * x).astype(np.float32)


def _inorm_np(x, g, be):
    m = x.mean((2, 3), keepdims=True, dtype=np.float32)
    v = x.var((2, 3), keepdims=True, dtype=np.float32)
    return ((x - m) / np.sqrt(v + np.float32(EPS)) * g[None, :, None, None]
            + be[None, :, None, None]).astype(np.float32)


def _backbone_np(imgs, w1, b1, g1, be1, w2, b2, g2, be2, w3, b3, g3, be3,
                 w4, b4, g4, be4, w5, b5, wgen_w, wgen_b, basis_w, ada_w, ada_b):
    b = imgs.shape[0]
    x = _resize_bilinear_np(imgs).astype(np.float32)
    for (w, bb, g, be) in ((w1, b1, g1, be1), (w2, b2, g2, be2),
                           (w3, b3, g3, be3), (w4, b4, g4, be4)):
        x = _inorm_np(_lrelu_np(_conv_s2_np(x, w, bb)), g, be)
    x = _lrelu_np(_conv_s2_np(x, w5, b5))                     # (b,128,8,8)
    x = x.reshape(b, 128, 2, 4, 2, 4).mean((3, 5), dtype=np.float32)
    x = x.reshape(b, 512).astype(np.float32)
    weights = x @ wgen_w + wgen_b                             # (b,3)
    luts = (weights @ basis_w).reshape(b, 3, V, V, V).astype(np.float32)
    logits = (x @ ada_w + ada_b).reshape(b, 3, V - 1).astype(np.float32)
    e = np.exp(logits - logits.max(-1, keepdims=True))
    intervals = (e / e.sum(-1, keepdims=True)).astype(np.float32)
    vertices = np.concatenate(
        [np.zeros((b, 3, 1), np.float32), np.cumsum(intervals, -1)], -1
    ).astype(np.float32)                                      # (b,3,V)
    return luts, vertices


def _build_tables(luts, vertices):
    """Per batch: E [32768, 32] f32 and vth [128, 96] f32."""
    b = luts.shape[0]
    ib, ig, ir = np.meshgrid(np.arange(32), np.arange(32), np.arange(32),
                             indexing="ij")
    etabs, vths = [], []
    for bi in range(b):
        E = np.zeros((32 * 32 * 32, 32), np.float32)
        col = 0
        for c in range(3):
            for db in (0, 1):
                for dg in (0, 1):
                    for dr in (0, 1):
                        E[:, col] = luts[bi, c, ib + db, ig + dg, ir + dr].ravel()
                        col += 1
        vtx = vertices[bi]  # (3, 33)
        # frac fields keyed by the cell's own (ir, ig, ib)
        for ci, axis in ((0, ir), (1, ig), (2, ib)):
            v0 = vtx[ci][axis].ravel()
            v1 = vtx[ci][axis.ravel() + 1]
            E[:, 24 + 2 * ci] = v0
            E[:, 25 + 2 * ci] = 1.0 / np.maximum(v1 - v0, 1e-10)
        etabs.append(E)
        # compare thresholds: v[1..32] per channel, replicated over partitions
        vth = np.zeros((128, 96), np.float32)
        for ci in range(3):
            vth[:, 32 * ci : 32 * ci + 32] = vtx[ci][1:33][None, :]
        vths.append(vth)
    return etabs, vths


# ------------------------------------------------------------- device kernel

_CACHED_NC = None


def _build_device_program():
    global _CACHED_NC
    if _CACHED_NC is not None:
        return _CACHED_NC
    import concourse.bass as bass
    import concourse.bacc as bacc
    import concourse.mybir as mybir
    import concourse.tile as tile
    from concourse.mybir import AluOpType as op

    T = 256            # pixel columns per tile
    NT = 32            # tiles: 32*256 = 8192 columns of 128 pixels
    f32 = mybir.dt.float32

    nc = bacc.Bacc("TRN2", target_bir_lowering=False, debug=False,
                   num_devices=8)
    img = nc.dram_tensor("img", [3, 128, 8192], f32, kind="ExternalInput").ap()
    etab = nc.dram_tensor("etab", [32768, 32], f32, kind="ExternalInput").ap()
    vth = nc.dram_tensor("vth", [128, 96], f32, kind="ExternalInput").ap()
    out = nc.dram_tensor("out", [3, 128, 8192], f32, kind="ExternalOutput").ap()

    with tile.TileContext(nc) as tc:
        with tc.tile_pool(name="cst", bufs=1) as cst, \
             tc.tile_pool(name="io", bufs=2) as io, \
             tc.tile_pool(name="gat", bufs=2) as gat, \
             tc.tile_pool(name="tmp", bufs=2) as tmp:
            vt = cst.tile([128, 96], f32)
            nc.sync.dma_start(vt[:], vth[:])

            with tc.For_i(0, NT, 1) as ti:
                x = []
                for c in range(3):
                    xc = io.tile([128, T], f32, tag=f"x{c}")
                    nc.sync.dma_start(xc[:], img[c, :, bass.ts(ti, T)])
                    # clip to [0,1]
                    nc.vector.tensor_scalar(out=xc[:], in0=xc[:],
                                            scalar1=0.0, scalar2=1.0,
                                            op0=op.max, op1=op.min)
                    x.append(xc)

                # searchsorted: acc_c = clip(sum_j (x_c >= v_j), 0, 31)
                accs = []
                for c in range(3):
                    a0 = tmp.tile([128, T], f32, tag=f"acc{c}a")
                    a1 = tmp.tile([128, T], f32, tag=f"acc{c}b")
                    nc.vector.tensor_scalar(out=a0[:], in0=x[c][:],
                                            scalar1=vt[:, 32 * c : 32 * c + 1],
                                            scalar2=0.0, op0=op.is_ge,
                                            op1=op.add)
                    cur, nxt = a0, a1
                    for j in range(1, 32):
                        nc.vector.scalar_tensor_tensor(
                            out=nxt[:], in0=x[c][:],
                            scalar=vt[:, 32 * c + j : 32 * c + j + 1],
                            in1=cur[:], op0=op.is_ge, op1=op.add)
                        cur, nxt = nxt, cur
                    nc.vector.tensor_scalar(out=cur[:], in0=cur[:],
                                            scalar1=31.0, scalar2=0.0,
                                            op0=op.min, op1=op.add)
                    accs.append(cur)

                # cell = (ib*32 + ig)*32 + ir   (r,g,b = ch 0,1,2 -> ir,ig,ib)
                cellf = tmp.tile([128, T], f32, tag="cellf")
                nc.vector.scalar_tensor_tensor(out=cellf[:], in0=accs[2][:],
                                               scalar=32.0, in1=accs[1][:],
                                               op0=op.mult, op1=op.add)
                nc.vector.scalar_tensor_tensor(out=cellf[:], in0=cellf[:],
                                               scalar=32.0, in1=accs[0][:],
                                               op0=op.mult, op1=op.add)
                celli = tmp.tile([128, T], mybir.dt.int32, tag="celli")
                nc.vector.tensor_copy(celli[:], cellf[:])

                # gather E rows, one pixel-column per indirect DMA
                g = gat.tile([128, T, 32], f32, tag="g")
                for m in range(T):
                    nc.gpsimd.indirect_dma_start(
                        out=g[:, m, :], out_offset=None, in_=etab[:],
                        in_offset=bass.IndirectOffsetOnAxis(
                            ap=celli[:, m : m + 1], axis=0))

                # fracs (exact, from gathered v0 / reciprocal fields)
                fr, omfr = [], []
                for c in range(3):
                    d = tmp.tile([128, T], f32, tag=f"d{c}")
                    nc.vector.tensor_sub(d[:], x[c][:], g[:, :, 24 + 2 * c])
                    nc.vector.tensor_mul(d[:], d[:], g[:, :, 25 + 2 * c])
                    nc.vector.tensor_scalar(out=d[:], in0=d[:], scalar1=0.0,
                                            scalar2=1.0, op0=op.max, op1=op.min)
                    o = tmp.tile([128, T], f32, tag=f"o{c}")
                    nc.vector.tensor_scalar(out=o[:], in0=d[:], scalar1=-1.0,
                                            scalar2=1.0, op0=op.mult, op1=op.add)
                    fr.append(d)
                    omfr.append(o)

                # trilinear reduce: lerp dr, then dg, then db
                for c in range(3):
                    L = []
                    for q in range(4):  # (db, dg) pairs
                        t0 = tmp.tile([128, T], f32, tag=f"L{q}")
                        t1 = tmp.tile([128, T], f32, tag=f"Lb{q}")
                        nc.vector.tensor_sub(t1[:], g[:, :, c * 8 + 2 * q + 1],
                                             g[:, :, c * 8 + 2 * q])
                        nc.vector.tensor_mul(t1[:], t1[:], fr[0][:])
                        nc.vector.tensor_add(t0[:], t1[:], g[:, :, c * 8 + 2 * q])
                        L.append(t0)
                    M = []
                    for h in range(2):  # db
                        nc.vector.tensor_sub(L[2 * h + 1][:], L[2 * h + 1][:],
                                             L[2 * h][:])
                        nc.vector.tensor_mul(L[2 * h + 1][:], L[2 * h + 1][:],
                                             fr[1][:])
                        nc.vector.tensor_add(L[2 * h][:], L[2 * h][:],
                                             L[2 * h + 1][:])
                        M.append(L[2 * h])
                    nc.vector.tensor_sub(M[1][:], M[1][:], M[0][:])
                    nc.vector.tensor_mul(M[1][:], M[1][:], fr[2][:])
                    oc = io.tile([128, T], f32, tag=f"oc{c}")
                    nc.vector.tensor_add(oc[:], M[0][:], M[1][:])
                    nc.sync.dma_start(out[c, :, bass.ts(ti, T)], oc[:])

    nc.compile()
    _CACHED_NC = nc
    return nc


# ------------------------------------------------------------------- kernel


def kernel(**inputs):
    imgs = np.asarray(inputs["imgs"], np.float32)
    b = imgs.shape[0]
    assert imgs.shape == (2, 3, 2048, 2048)

    luts, vertices = _backbone_np(
        imgs,
        *[np.asarray(inputs[k], np.float32) for k in
          ("w1", "b1", "g1", "be1", "w2", "b2", "g2", "be2",
           "w3", "b3", "g3", "be3", "w4", "b4", "g4", "be4", "w5", "b5",
           "wgen_w", "wgen_b", "basis_w", "ada_w", "ada_b")])
    etabs, vths = _build_tables(luts, vertices)

    nc = _build_device_program()
    from concourse.bass_utils import run_bass_kernel_spmd

    in_maps = []
    for core in range(8):
        bi, blk = core // 4, core % 4
        sl = imgs[bi, :, 512 * blk : 512 * (blk + 1), :]       # [3,512,2048]
        in_maps.append({
            "img": np.ascontiguousarray(sl).reshape(3, 128, 8192),
            "etab": etabs[bi],
            "vth": vths[bi],
        })
    res = run_bass_kernel_spmd(nc, in_maps, core_ids=list(range(8)))

    outp = np.zeros((2, 3, 2048, 2048), np.float32)
    for core in range(8):
        bi, blk = core // 4, core % 4
        outp[bi, :, 512 * blk : 512 * (blk + 1), :] = (
            res.results[core]["out"].reshape(3, 512, 2048))
    return outp
